# revision 7
# baseline (speedup 1.0000x reference)
"""CustomMultiMarginLoss (p=1, margin=1.0, mean reduction) on 8 NeuronCores.

Math: loss = mean_b( sum_{c != t_b} max(0, 1 - (x[b,t_b] - x[b,c])) )
The excluded target column would contribute exactly relu(1) = 1, so
    loss = (1/B) * sum_b sum_c relu(x[b,c] + (1 - x[b,t_b])) - 1
which turns the whole problem into a streaming relu-with-per-row-bias plus a
row reduction: one fused instruction per tile (ACT `activation(Relu, bias,
accum_out)` / DVE `scalar_tensor_tensor(add, max, accum_out)`).

Sharding: data parallel over the batch dim. Core k owns rows
[k*1024, (k+1)*1024), processed as 8 blocks of 128 rows (rows on SBUF
partitions), streaming the 32000-wide class dim in 4000-wide chunks (2 MiB
DMAs). Per-(block, chunk) row-sums land in accumulator columns; the host sums
the 8 per-core [128, 64] partials in float64 (the "all-reduce") and applies
the /B and -1 corrections.
"""

import numpy as np

B = 8192
C = 32000
NCORES = 8
ROWS_PER_CORE = B // NCORES  # 1024
P = 128
NBLK = ROWS_PER_CORE // P  # 8 blocks of 128 rows per core
W = 4000  # chunk width: 128 * 4000 * 4B = 2 MiB per DMA
NCHUNK = C // W  # 8
HALF = NBLK * NCHUNK // 2  # 32 accumulator columns per engine

_CACHE: dict = {}


def _build_program():
    import concourse.bacc as bacc
    import concourse.mybir as mybir
    from concourse.tile import TileContext

    f32 = mybir.dt.float32
    nc = bacc.Bacc(None, target_bir_lowering=False)
    inp = nc.dram_tensor("inp", [ROWS_PER_CORE, C], f32, kind="ExternalInput")
    # Last column is a host-supplied 0.0 (used as DVE max-operand), so no
    # device-side memset is needed.
    bias = nc.dram_tensor("bias", [P, NBLK + 1], f32, kind="ExternalInput")
    out = nc.dram_tensor("out", [P, 2 * HALF], f32, kind="ExternalOutput")

    inp_r = inp.rearrange("(nb p) c -> nb p c", p=P)  # [NBLK, 128, C]

    with TileContext(nc) as tc:
        with (
            tc.tile_pool(name="x", bufs=8) as xpool,
            tc.tile_pool(name="misc", bufs=1) as misc,
        ):
            bias_t = misc.tile([P, NBLK + 1], f32)
            nc.sync.dma_start(bias_t[:], bias[:, :])
            zeros = bias_t[:, NBLK : NBLK + 1]
            acc_a = misc.tile([P, HALF], f32)  # ACT-written row sums
            acc_v = misc.tile([P, HALF], f32)  # DVE-written row sums
            dummy_a = misc.tile([P, W], f32)
            dummy_v = misc.tile([P, W], f32)

            for j in range(NBLK):
                bj = bias_t[:, j : j + 1]
                for i in range(NCHUNK):
                    xt = xpool.tile([P, W], f32)
                    nc.sync.dma_start(xt[:], inp_r[j, :, i * W : (i + 1) * W])
                    col = j * (NCHUNK // 2) + i // 2
                    if i % 2 == 0:
                        nc.scalar.activation(
                            dummy_a[:],
                            xt[:],
                            mybir.ActivationFunctionType.Relu,
                            bias=bj,
                            scale=1.0,
                            accum_out=acc_a[:, col : col + 1],
                        )
                    else:
                        nc.vector.scalar_tensor_tensor(
                            out=dummy_v[:],
                            in0=xt[:],
                            scalar=bj,
                            in1=zeros.broadcast_to((P, W)),
                            op0=mybir.AluOpType.add,
                            op1=mybir.AluOpType.max,
                            accum_out=acc_v[:, col : col + 1],
                        )

            nc.sync.dma_start(out[:, :HALF], acc_a[:])
            nc.sync.dma_start(out[:, HALF:], acc_v[:])

    nc.finalize()
    return nc


def _get_program():
    if "nc" not in _CACHE:
        _CACHE["nc"] = _build_program()
    return _CACHE["nc"]


def _make_in_maps(x: np.ndarray, t: np.ndarray) -> list:
    # Per-row correct-class score and relu bias, computed during shard prep.
    correct = x[np.arange(B), t]  # [B] f32
    bias_full = (np.float32(1.0) - correct).astype(np.float32)

    in_maps = []
    for k in range(NCORES):
        r0 = k * ROWS_PER_CORE
        shard = x[r0 : r0 + ROWS_PER_CORE]
        bias_core = np.zeros((P, NBLK + 1), dtype=np.float32)
        bias_core[:, :NBLK] = bias_full[r0 : r0 + ROWS_PER_CORE].reshape(NBLK, P).T
        in_maps.append({"inp": shard, "bias": bias_core})
    return in_maps


def kernel(input: np.ndarray, target: np.ndarray, _results_out: list | None = None):
    from concourse.bass_utils import run_bass_kernel_spmd

    x = np.ascontiguousarray(np.asarray(input, dtype=np.float32))
    t = np.asarray(target).astype(np.int64)

    nc = _get_program()
    in_maps = _make_in_maps(x, t)

    res = run_bass_kernel_spmd(nc, in_maps, core_ids=list(range(NCORES)))
    if _results_out is not None:
        _results_out.append(res)

    total = np.float64(0.0)
    for k in range(NCORES):
        total += res.results[k]["out"].astype(np.float64).sum()

    loss = total / np.float64(B) - np.float64(1.0)
    return np.array(loss, dtype=np.float32)


# revision 9
# speedup vs baseline: 72848.2293x; 72848.2293x over previous
"""CustomMultiMarginLoss (p=1, margin=1.0, mean reduction) on 8 NeuronCores.

Math: loss = mean_b( sum_{c != t_b} max(0, 1 - (x[b,t_b] - x[b,c])) )
The excluded target column would contribute exactly relu(1) = 1, so
    loss = (1/B) * sum_b sum_c relu(x[b,c] + (1 - x[b,t_b])) - 1
which turns the whole problem into a streaming relu-with-per-row-bias plus a
row reduction: one fused instruction per tile (ACT `activation(Relu, bias,
accum_out)` / DVE `scalar_tensor_tensor(add, max, accum_out)`).

Sharding: data parallel over the batch dim. Core k owns rows
[k*1024, (k+1)*1024), processed as 8 blocks of 128 rows (rows on SBUF
partitions), streaming the 32000-wide class dim in 4000-wide chunks (2 MiB
DMAs). Per-(block, chunk) row-sums land in accumulator columns; the host sums
the 8 per-core [128, 64] partials in float64 (the "all-reduce") and applies
the /B and -1 corrections.
"""

import numpy as np

B = 8192
C = 32000
NCORES = 8
ROWS_PER_CORE = B // NCORES  # 1024
P = 128
NBLK = ROWS_PER_CORE // P  # 8 blocks of 128 rows per core
W = 4000  # chunk width: 128 * 4000 * 4B = 2 MiB per DMA
NCHUNK = C // W  # 8
HALF = NBLK * NCHUNK // 2  # 32 accumulator columns per engine

_CACHE: dict = {}


def _build_program(repeat: int = 1):
    # repeat>1 duplicates the streaming body (re-reading the same input) —
    # used only for benchmarking to separate HW exec time from dispatch
    # overhead via the slope of time vs repeat.
    import concourse.bacc as bacc
    import concourse.mybir as mybir
    from concourse.tile import TileContext

    f32 = mybir.dt.float32
    nc = bacc.Bacc(None, target_bir_lowering=False)
    inp = nc.dram_tensor("inp", [ROWS_PER_CORE, C], f32, kind="ExternalInput")
    # Last column is a host-supplied 0.0 (used as DVE max-operand), so no
    # device-side memset is needed.
    bias = nc.dram_tensor("bias", [P, NBLK + 1], f32, kind="ExternalInput")
    out = nc.dram_tensor("out", [P, 2 * HALF], f32, kind="ExternalOutput")

    inp_r = inp.rearrange("(nb p) c -> nb p c", p=P)  # [NBLK, 128, C]

    with TileContext(nc) as tc:
        with (
            tc.tile_pool(name="x", bufs=8) as xpool,
            tc.tile_pool(name="misc", bufs=1) as misc,
        ):
            bias_t = misc.tile([P, NBLK + 1], f32)
            nc.sync.dma_start(bias_t[:], bias[:, :])
            zeros = bias_t[:, NBLK : NBLK + 1]
            acc_a = misc.tile([P, HALF], f32)  # ACT-written row sums
            acc_v = misc.tile([P, HALF], f32)  # DVE-written row sums
            dummy_a = misc.tile([P, W], f32)
            dummy_v = misc.tile([P, W], f32)

            for j in range(NBLK * repeat):
                j = j % NBLK
                bj = bias_t[:, j : j + 1]
                for i in range(NCHUNK):
                    xt = xpool.tile([P, W], f32)
                    nc.sync.dma_start(xt[:], inp_r[j, :, i * W : (i + 1) * W])
                    col = j * (NCHUNK // 2) + i // 2
                    if i % 2 == 0:
                        nc.scalar.activation(
                            dummy_a[:],
                            xt[:],
                            mybir.ActivationFunctionType.Relu,
                            bias=bj,
                            scale=1.0,
                            accum_out=acc_a[:, col : col + 1],
                        )
                    else:
                        nc.vector.scalar_tensor_tensor(
                            out=dummy_v[:],
                            in0=xt[:],
                            scalar=bj,
                            in1=zeros.broadcast_to((P, W)),
                            op0=mybir.AluOpType.add,
                            op1=mybir.AluOpType.max,
                            accum_out=acc_v[:, col : col + 1],
                        )

            nc.sync.dma_start(out[:, :HALF], acc_a[:])
            nc.sync.dma_start(out[:, HALF:], acc_v[:])

    nc.finalize()
    return nc


def _get_program():
    if "nc" not in _CACHE:
        _CACHE["nc"] = _build_program()
    return _CACHE["nc"]


def _make_in_maps(x: np.ndarray, t: np.ndarray) -> list:
    # Per-row correct-class score and relu bias, computed during shard prep.
    correct = x[np.arange(B), t]  # [B] f32
    bias_full = (np.float32(1.0) - correct).astype(np.float32)

    in_maps = []
    for k in range(NCORES):
        r0 = k * ROWS_PER_CORE
        shard = x[r0 : r0 + ROWS_PER_CORE]
        bias_core = np.zeros((P, NBLK + 1), dtype=np.float32)
        bias_core[:, :NBLK] = bias_full[r0 : r0 + ROWS_PER_CORE].reshape(NBLK, P).T
        in_maps.append({"inp": shard, "bias": bias_core})
    return in_maps


def kernel(input: np.ndarray, target: np.ndarray, _results_out: list | None = None):
    from concourse.bass_utils import run_bass_kernel_spmd

    x = np.ascontiguousarray(np.asarray(input, dtype=np.float32))
    t = np.asarray(target).astype(np.int64)

    nc = _get_program()
    in_maps = _make_in_maps(x, t)

    res = run_bass_kernel_spmd(nc, in_maps, core_ids=list(range(NCORES)))
    if _results_out is not None:
        _results_out.append(res)

    total = np.float64(0.0)
    for k in range(NCORES):
        total += res.results[k]["out"].astype(np.float64).sum()

    loss = total / np.float64(B) - np.float64(1.0)
    return np.array(loss, dtype=np.float32)


# revision 15
# speedup vs baseline: 76706.4259x; 1.0530x over previous
"""CustomMultiMarginLoss (p=1, margin=1.0, mean reduction) on 8 NeuronCores.

Math: loss = mean_b( sum_{c != t_b} max(0, 1 - (x[b,t_b] - x[b,c])) )
The excluded target column would contribute exactly relu(1) = 1, so
    loss = (1/B) * sum_b sum_c relu(x[b,c] + (1 - x[b,t_b])) - 1
which turns the whole problem into a streaming relu-with-per-row-bias plus a
row reduction: one fused instruction per tile (ACT `activation(Relu, bias,
accum_out)` / DVE `scalar_tensor_tensor(add, max, accum_out)`).

Sharding: data parallel over the batch dim. Core k owns rows
[k*1024, (k+1)*1024), processed as 8 blocks of 128 rows (rows on SBUF
partitions), streaming the 32000-wide class dim in 3200-wide chunks (1.56 MiB
HWDGE DMAs, deep-buffered). Per-(block, chunk) row-sums land in accumulator
columns; the host sums the 8 per-core [128, 80] partials in float64 (the
"all-reduce") and applies the /B and -1 corrections.
"""

import numpy as np

B = 8192
C = 32000
NCORES = 8
ROWS_PER_CORE = B // NCORES  # 1024
P = 128
NBLK = ROWS_PER_CORE // P  # 8 blocks of 128 rows per core
W = 3200  # chunk width: 128 * 3200 * 4B = 1.56 MiB per DMA
NCHUNK = C // W  # 10
BUFS = 10  # x-tile slots: 10 * 12.8 KiB = 128 KiB/partition

_CACHE: dict = {}


def _build_program(repeat: int = 1, w: int = W, bufs: int = BUFS):
    # repeat>1 duplicates the streaming body (re-reading the same input) —
    # used only for benchmarking to separate HW exec time from dispatch
    # overhead via the slope of time vs repeat. w/bufs are benchmarking knobs
    # for the chunk width and x-tile double-buffer depth.
    import concourse.bacc as bacc
    import concourse.mybir as mybir
    from concourse.tile import TileContext

    nchunk = C // w
    ncol = NBLK * nchunk  # one accumulator column per (block, chunk)

    f32 = mybir.dt.float32
    nc = bacc.Bacc(None, target_bir_lowering=False)
    inp = nc.dram_tensor("inp", [ROWS_PER_CORE, C], f32, kind="ExternalInput")
    # Last column is a host-supplied 0.0 (used as DVE max-operand), so no
    # device-side memset is needed.
    bias = nc.dram_tensor("bias", [P, NBLK + 1], f32, kind="ExternalInput")
    out = nc.dram_tensor("out", [P, ncol], f32, kind="ExternalOutput")

    inp_r = inp.rearrange("(nb p) c -> nb p c", p=P)  # [NBLK, 128, C]

    with TileContext(nc) as tc:
        with (
            tc.tile_pool(name="x", bufs=bufs) as xpool,
            tc.tile_pool(name="misc", bufs=1) as misc,
        ):
            bias_t = misc.tile([P, NBLK + 1], f32)
            nc.sync.dma_start(bias_t[:], bias[:, :])
            zeros = bias_t[:, NBLK : NBLK + 1]
            acc = misc.tile([P, ncol], f32)  # even cols ACT, odd cols DVE
            dummy_a = misc.tile([P, w], f32)
            dummy_v = misc.tile([P, w], f32)

            for j in range(NBLK * repeat):
                j = j % NBLK
                bj = bias_t[:, j : j + 1]
                for i in range(nchunk):
                    xt = xpool.tile([P, w], f32)
                    nc.sync.dma_start(xt[:], inp_r[j, :, i * w : (i + 1) * w])
                    col = j * nchunk + i
                    if i % 2 == 0:
                        nc.scalar.activation(
                            dummy_a[:],
                            xt[:],
                            mybir.ActivationFunctionType.Relu,
                            bias=bj,
                            scale=1.0,
                            accum_out=acc[:, col : col + 1],
                        )
                    else:
                        nc.vector.scalar_tensor_tensor(
                            out=dummy_v[:],
                            in0=xt[:],
                            scalar=bj,
                            in1=zeros.broadcast_to((P, w)),
                            op0=mybir.AluOpType.add,
                            op1=mybir.AluOpType.max,
                            accum_out=acc[:, col : col + 1],
                        )

            nc.sync.dma_start(out[:], acc[:])

    nc.finalize()
    return nc


def _get_program():
    if "nc" not in _CACHE:
        _CACHE["nc"] = _build_program()
    return _CACHE["nc"]


def _make_in_maps(x: np.ndarray, t: np.ndarray) -> list:
    # Per-row correct-class score and relu bias, computed during shard prep.
    correct = x[np.arange(B), t]  # [B] f32
    bias_full = (np.float32(1.0) - correct).astype(np.float32)

    in_maps = []
    for k in range(NCORES):
        r0 = k * ROWS_PER_CORE
        shard = x[r0 : r0 + ROWS_PER_CORE]
        bias_core = np.zeros((P, NBLK + 1), dtype=np.float32)
        bias_core[:, :NBLK] = bias_full[r0 : r0 + ROWS_PER_CORE].reshape(NBLK, P).T
        in_maps.append({"inp": shard, "bias": bias_core})
    return in_maps


def kernel(input: np.ndarray, target: np.ndarray, _results_out: list | None = None):
    from concourse.bass_utils import run_bass_kernel_spmd

    x = np.ascontiguousarray(np.asarray(input, dtype=np.float32))
    t = np.asarray(target).astype(np.int64)

    nc = _get_program()
    in_maps = _make_in_maps(x, t)

    res = run_bass_kernel_spmd(nc, in_maps, core_ids=list(range(NCORES)))
    if _results_out is not None:
        _results_out.append(res)

    total = np.float64(0.0)
    for k in range(NCORES):
        total += res.results[k]["out"].astype(np.float64).sum()

    loss = total / np.float64(B) - np.float64(1.0)
    return np.array(loss, dtype=np.float32)


# revision 17
# speedup vs baseline: 76910.3028x; 1.0027x over previous
"""CustomMultiMarginLoss (p=1, margin=1.0, mean reduction) on 8 NeuronCores.

Math: loss = mean_b( sum_{c != t_b} max(0, 1 - (x[b,t_b] - x[b,c])) )
The excluded target column would contribute exactly relu(1) = 1, so
    loss = (1/B) * sum_b sum_c relu(x[b,c] + (1 - x[b,t_b])) - 1
which turns the whole problem into a streaming relu-with-per-row-bias plus a
row reduction: one fused instruction per tile (ACT `activation(Relu, bias,
accum_out)` / DVE `scalar_tensor_tensor(add, max, accum_out)`).

Sharding: data parallel over the batch dim. Core k owns rows
[k*1024, (k+1)*1024), processed as 8 blocks of 128 rows (rows on SBUF
partitions), streaming the 32000-wide class dim in 4000-wide chunks (2 MiB
HWDGE DMAs, deep-buffered). Per-(block, chunk) row-sums land in accumulator
columns; the host sums the 8 per-core [128, 64] partials in float64 (the
"all-reduce") and applies the /B and -1 corrections.
"""

import numpy as np

B = 8192
C = 32000
NCORES = 8
ROWS_PER_CORE = B // NCORES  # 1024
P = 128
NBLK = ROWS_PER_CORE // P  # 8 blocks of 128 rows per core
W = 4000  # chunk width: 128 * 4000 * 4B = 2 MiB per DMA
NCHUNK = C // W  # 8
BUFS = 10  # x-tile slots: 10 * 16 KiB = 160 KiB/partition

_CACHE: dict = {}


def _build_program(repeat: int = 1, w: int = W, bufs: int = BUFS):
    # repeat>1 duplicates the streaming body (re-reading the same input) —
    # used only for benchmarking to separate HW exec time from dispatch
    # overhead via the slope of time vs repeat. w/bufs are benchmarking knobs
    # for the chunk width and x-tile double-buffer depth.
    import concourse.bacc as bacc
    import concourse.mybir as mybir
    from concourse.tile import TileContext

    nchunk = C // w
    ncol = NBLK * nchunk  # one accumulator column per (block, chunk)

    f32 = mybir.dt.float32
    nc = bacc.Bacc(None, target_bir_lowering=False)
    inp = nc.dram_tensor("inp", [ROWS_PER_CORE, C], f32, kind="ExternalInput")
    # Last column is a host-supplied 0.0 (used as DVE max-operand), so no
    # device-side memset is needed.
    bias = nc.dram_tensor("bias", [P, NBLK + 1], f32, kind="ExternalInput")
    out = nc.dram_tensor("out", [P, ncol], f32, kind="ExternalOutput")

    inp_r = inp.rearrange("(nb p) c -> nb p c", p=P)  # [NBLK, 128, C]

    with TileContext(nc) as tc:
        with (
            tc.tile_pool(name="x", bufs=bufs) as xpool,
            tc.tile_pool(name="misc", bufs=1) as misc,
        ):
            bias_t = misc.tile([P, NBLK + 1], f32)
            nc.sync.dma_start(bias_t[:], bias[:, :])
            zeros = bias_t[:, NBLK : NBLK + 1]
            acc = misc.tile([P, ncol], f32)  # even cols ACT, odd cols DVE
            dummy_a = misc.tile([P, w], f32)
            dummy_v = misc.tile([P, w], f32)

            for j in range(NBLK * repeat):
                j = j % NBLK
                bj = bias_t[:, j : j + 1]
                for i in range(nchunk):
                    xt = xpool.tile([P, w], f32)
                    nc.sync.dma_start(xt[:], inp_r[j, :, i * w : (i + 1) * w])
                    col = j * nchunk + i
                    if i % 2 == 0:
                        nc.scalar.activation(
                            dummy_a[:],
                            xt[:],
                            mybir.ActivationFunctionType.Relu,
                            bias=bj,
                            scale=1.0,
                            accum_out=acc[:, col : col + 1],
                        )
                    else:
                        nc.vector.scalar_tensor_tensor(
                            out=dummy_v[:],
                            in0=xt[:],
                            scalar=bj,
                            in1=zeros.broadcast_to((P, w)),
                            op0=mybir.AluOpType.add,
                            op1=mybir.AluOpType.max,
                            accum_out=acc[:, col : col + 1],
                        )

            nc.sync.dma_start(out[:], acc[:])

    nc.finalize()
    return nc


def _get_program():
    if "nc" not in _CACHE:
        _CACHE["nc"] = _build_program()
    return _CACHE["nc"]


def _make_in_maps(x: np.ndarray, t: np.ndarray) -> list:
    # Per-row correct-class score and relu bias, computed during shard prep.
    correct = x[np.arange(B), t]  # [B] f32
    bias_full = (np.float32(1.0) - correct).astype(np.float32)

    in_maps = []
    for k in range(NCORES):
        r0 = k * ROWS_PER_CORE
        shard = x[r0 : r0 + ROWS_PER_CORE]
        bias_core = np.zeros((P, NBLK + 1), dtype=np.float32)
        bias_core[:, :NBLK] = bias_full[r0 : r0 + ROWS_PER_CORE].reshape(NBLK, P).T
        in_maps.append({"inp": shard, "bias": bias_core})
    return in_maps


def kernel(input: np.ndarray, target: np.ndarray, _results_out: list | None = None):
    from concourse.bass_utils import run_bass_kernel_spmd

    x = np.ascontiguousarray(np.asarray(input, dtype=np.float32))
    t = np.asarray(target).astype(np.int64)

    nc = _get_program()
    in_maps = _make_in_maps(x, t)

    res = run_bass_kernel_spmd(nc, in_maps, core_ids=list(range(NCORES)))
    if _results_out is not None:
        _results_out.append(res)

    total = np.float64(0.0)
    for k in range(NCORES):
        total += res.results[k]["out"].astype(np.float64).sum()

    loss = total / np.float64(B) - np.float64(1.0)
    return np.array(loss, dtype=np.float32)
